# revision 2
# baseline (speedup 1.0000x reference)
"""Multi-head causal attention with RoPE on 8 Trainium2 NeuronCores.

Sharding: tensor-parallel over heads x data-parallel over batch.
Core c handles batch b = c//4 and heads [4*(c%4), 4*(c%4)+4) (Hl=256 of Hd=1024).
Each core computes q/k/v projections for its head slice (column-split Wq/Wk/Wv),
RoPE, causal softmax attention, and a partial output projection (row-split Wo).
The host sums the 4 partial outputs per batch (the "all-reduce").

Device layouts (per core, S=2048, E=1024, Hl=256, D=64):
  xT   [E, S]    x transposed (host-side) so E rides the partition dim
  qT/kT slabs [128, S] x2: partitions = 2 heads x 64 dims, free = seq
  v    16 tiles [128, 260]: partitions = seq chunk, free = 4 heads x (64 dims + ones col)
  scores computed transposed (keys on partitions), softmax Z via ones-column of v,
  normalization by 1/Z broadcast via a DRAM-roundtrip partition-broadcast DMA.

All matmuls run in float32r (single-pass PE, ~1.5e-4 rel err measured on HW).
"""
import sys

sys.path.insert(0, "/opt/trn_rl_repo")
import numpy as np  # noqa: E402

N_HEADS = 16
B, S, E, HD = 2, 2048, 1024, 1024
D = HD // N_HEADS  # 64
HPC = 4            # heads per core
HL = HPC * D       # 256
NCORES = 8
ROPE_BASE = 10000.0

_built = None


def _build_nc():
    import concourse.tile as tile
    from concourse import bacc, mybir

    F32 = mybir.dt.float32
    F32R = mybir.dt.float32r
    Exp = mybir.ActivationFunctionType.Exp
    is_ge = mybir.AluOpType.is_ge

    def ts(i, n):
        import concourse.bass as bass
        return bass.ts(i, n)

    nc = bacc.Bacc("TRN2", target_bir_lowering=False, debug=False)
    xT_d = nc.dram_tensor("xT", [E, S], F32, kind="ExternalInput").ap()
    wq_d = nc.dram_tensor("wq", [E, HL], F32, kind="ExternalInput").ap()
    wk_d = nc.dram_tensor("wk", [E, HL], F32, kind="ExternalInput").ap()
    wv_d = nc.dram_tensor("wv", [E, HL], F32, kind="ExternalInput").ap()
    wo_d = nc.dram_tensor("wo", [HL, E], F32, kind="ExternalInput").ap()
    cos_d = nc.dram_tensor("cosx", [128, S], F32, kind="ExternalInput").ap()
    sin_d = nc.dram_tensor("sinx", [128, S], F32, kind="ExternalInput").ap()
    out_d = nc.dram_tensor("out", [S, E], F32, kind="ExternalOutput").ap()
    zscr_d = nc.dram_tensor("zscr", [HPC, S], F32).ap()  # internal scratch

    ECH = E // 128   # 8 e-chunks
    SCH = S // 128   # 16 seq chunks
    SB = S // 512    # 4 seq blocks
    swap_mask = []
    for i in range(16):
        swap_mask += [2 * i + 1, 2 * i]

    with tile.TileContext(nc) as tc:
        with (
            tc.tile_pool(name="persist", bufs=1) as pp,
            tc.tile_pool(name="evict", bufs=3) as ev,
        ):
            # persistent tiles
            qT = [pp.tile([128, S], F32R, tag=f"qT{c}", name=f"qT{c}") for c in range(2)]
            kT = [pp.tile([128, S], F32R, tag=f"kT{c}", name=f"kT{c}") for c in range(2)]
            vt = [pp.tile([128, HPC * (D + 1)], F32R, tag=f"v{t}", name=f"v{t}") for t in range(SCH)]
            oT = [pp.tile([128, S], F32R, tag=f"oT{c}", name=f"oT{c}") for c in range(2)]
            cosx = pp.tile([128, S], F32R, tag="cosx", name="cosx")
            sinx = pp.tile([128, S], F32R, tag="sinx", name="sinx")
            wo_t = pp.tile([128, 2, E], F32R, tag="wo", name="wo")

            nc.sync.dma_start(out=cosx[:], in_=cos_d.bitcast(F32R))
            nc.sync.dma_start(out=sinx[:], in_=sin_d.bitcast(F32R))
            nc.sync.dma_start(
                out=wo_t[:], in_=wo_d.rearrange("(c p) e -> p c e", p=128).bitcast(F32R)
            )

            # ---------------- Phase B: projections + RoPE ----------------
            with (
                tc.tile_pool(name="bx", bufs=1) as bx,
                tc.tile_pool(name="bswp", bufs=2) as bswp,
                tc.tile_pool(name="bps", bufs=4, space="PSUM") as bps,
            ):
                xt = []
                for e in range(ECH):
                    xe = bx.tile([128, S], F32R, tag=f"x{e}", name=f"x{e}")
                    nc.sync.dma_start(
                        out=xe[:], in_=xT_d[e * 128:(e + 1) * 128, :].bitcast(F32R)
                    )
                    xt.append(xe)
                wq_t = bx.tile([128, ECH, HL], F32R, tag="wq", name="wq")
                wk_t = bx.tile([128, ECH, HL], F32R, tag="wk", name="wk")
                wv_t = bx.tile([128, ECH, HL], F32R, tag="wv", name="wv")
                for w_d_, w_t_ in ((wq_d, wq_t), (wk_d, wk_t), (wv_d, wv_t)):
                    nc.sync.dma_start(
                        out=w_t_[:],
                        in_=w_d_.rearrange("(c p) m -> p c m", p=128).bitcast(F32R),
                    )

                # q/k projections -> transposed slabs
                for w_t_, dest in ((wq_t, qT), (wk_t, kT)):
                    for m in range(2):
                        for j in range(SB):
                            ps = bps.tile([128, 512], F32, tag="mm", name="mm")
                            for e in range(ECH):
                                nc.tensor.matmul(
                                    ps[:],
                                    w_t_[:, e, m * 128:(m + 1) * 128],
                                    xt[e][:, ts(j, 512)],
                                    start=(e == 0),
                                    stop=(e == ECH - 1),
                                )
                            nc.vector.tensor_copy(
                                out=dest[m][:, ts(j, 512)], in_=ps[:]
                            )
                # v projection -> seq-partition tiles with ones column
                for t in range(SCH):
                    nc.vector.memset(
                        vt[t].rearrange("p (h c) -> p h c", c=D + 1)[:, :, D:D + 1]
                        .bitcast(F32),
                        1.0,
                    )
                    ps = bps.tile([128, HL], F32, tag="mm", name="mmv")
                    for e in range(ECH):
                        nc.tensor.matmul(
                            ps[:],
                            xt[e][:, ts(t, 128)],
                            wv_t[:, e, :],
                            start=(e == 0),
                            stop=(e == ECH - 1),
                        )
                    nc.vector.tensor_copy(
                        out=vt[t].rearrange("p (h c) -> p h c", c=D + 1)[:, :, 0:D],
                        in_=ps.rearrange("p (h c) -> p h c", c=D),
                    )
                # RoPE on q/k slabs (interleaved pairs ride the partition dim)
                for dest in (qT, kT):
                    for c in range(2):
                        sw = bswp.tile([128, S], F32R, tag="swp", name="swp")
                        nc.vector.stream_shuffle(
                            out=sw[:].bitcast(F32),
                            in_=dest[c][:].bitcast(F32),
                            mask=swap_mask,
                        )
                        nc.vector.tensor_mul(out=sw[:], in0=sw[:], in1=sinx[:])
                        nc.vector.tensor_mul(
                            out=dest[c][:], in0=dest[c][:], in1=cosx[:]
                        )
                        nc.vector.tensor_add(
                            out=dest[c][:], in0=dest[c][:], in1=sw[:]
                        )

            # ---------------- Phase C: attention ----------------
            with (
                tc.tile_pool(name="cexp", bufs=3) as cexp,
                tc.tile_pool(name="cz", bufs=2) as cz,
                tc.tile_pool(name="crb", bufs=2) as crb,
                tc.tile_pool(name="csc", bufs=2, space="PSUM") as csc,
                tc.tile_pool(name="cpv", bufs=2, space="PSUM") as cpv,
            ):
                for h in range(HPC):
                    c = h // 2
                    base = (h % 2) * 64
                    qh = qT[c][base:base + 64, :]
                    kh = kT[c][base:base + 64, :]
                    vh = [
                        vt[t].rearrange("p (h c) -> p h c", c=D + 1)[:, h, :]
                        for t in range(SCH)
                    ]
                    zst = cz.tile([65, S], F32, tag="zst", name="zst")
                    for j in range(SB):
                        nt = 4 * (j + 1)
                        pv = cpv.tile([65, 512], F32, tag="pv", name="pv")
                        for tp in range(nt // 2):
                            sc = csc.tile([128, 1024], F32, tag="sc", name="sc")
                            for half in range(2):
                                t = 2 * tp + half
                                nc.tensor.matmul(
                                    sc[:, ts(half, 512)],
                                    kh[:, ts(t, 128)],
                                    qh[:, ts(j, 512)],
                                    start=True,
                                    stop=True,
                                )
                            ex = cexp.tile([128, 1024], F32R, tag="ex", name="ex")
                            nc.scalar.activation(
                                out=ex[:], in_=sc[:], func=Exp, scale=0.125
                            )
                            for half in range(2):
                                t = 2 * tp + half
                                if t >= nt - 4:  # diagonal chunk: causal mask
                                    nc.gpsimd.affine_select(
                                        out=ex[:, ts(half, 512)],
                                        in_=ex[:, ts(half, 512)],
                                        compare_op=is_ge,
                                        fill=0.0,
                                        base=(j * 512 - t * 128),
                                        channel_multiplier=-1,
                                        pattern=[[1, 512]],
                                    )
                                nc.tensor.matmul(
                                    pv[:],
                                    vh[t],
                                    ex[:, ts(half, 512)],
                                    start=(t == 0),
                                    stop=(t == nt - 1),
                                )
                        nc.vector.tensor_copy(
                            out=oT[c][base:base + 64, ts(j, 512)], in_=pv[0:64, :]
                        )
                        nc.vector.reciprocal(
                            out=zst[64:65, ts(j, 512)], in_=pv[64:65, :]
                        )
                    nc.sync.dma_start(out=zscr_d[h, :], in_=zst[64:65, :])
                # normalize: broadcast 1/Z back across the 64 dim-partitions
                for c in range(2):
                    rb = crb.tile([128, S], F32R, tag="rb", name="rb")
                    for half in range(2):
                        h = 2 * c + half
                        nc.sync.dma_start(
                            out=rb[half * 64:(half + 1) * 64, :],
                            in_=zscr_d[h:h + 1, :].bitcast(F32R).to_broadcast((64, S)),
                        )
                    nc.vector.tensor_mul(out=oT[c][:], in0=oT[c][:], in1=rb[:])

            # ---------------- Phase D: output projection (row-split Wo) --------
            with tc.tile_pool(name="dps", bufs=4, space="PSUM") as dps:
                for t in range(SCH):
                    for n in range(2):
                        ps = dps.tile([128, 512], F32, tag="wo", name="wops")
                        for c in range(2):
                            nc.tensor.matmul(
                                ps[:],
                                oT[c][:, ts(t, 128)],
                                wo_t[:, c, ts(n, 512)],
                                start=(c == 0),
                                stop=(c == 1),
                            )
                        ot = ev.tile([128, 512], F32, tag="out", name="oev")
                        nc.vector.tensor_copy(out=ot[:], in_=ps[:])
                        nc.sync.dma_start(
                            out=out_d[ts(t, 128), ts(n, 512)], in_=ot[:]
                        )

    nc.compile()
    return nc


def _rope_tables():
    iexp = np.arange(0, D, 2, dtype=np.float32) / np.float32(D)
    inv_freq = np.reciprocal(np.power(np.float32(ROPE_BASE), iexp))  # (32,) f32
    ang = np.arange(S, dtype=np.float32)[:, None] * inv_freq[None, :]  # (S, 32)
    cos = np.cos(ang).astype(np.float32)  # (S, 32)
    sin = np.sin(ang).astype(np.float32)
    cosx = np.empty((64, S), dtype=np.float32)
    sinx = np.empty((64, S), dtype=np.float32)
    cosx[0::2] = cos.T
    cosx[1::2] = cos.T
    sinx[0::2] = -sin.T
    sinx[1::2] = sin.T
    return np.tile(cosx, (2, 1)), np.tile(sinx, (2, 1))  # (128, S) each


def get_nc():
    global _built
    if _built is None:
        _built = _build_nc()
    return _built


def make_in_maps(x, Wq, Wk, Wv, Wo):
    cosx, sinx = _rope_tables()
    in_maps = []
    for c in range(NCORES):
        b, g = c // 4, c % 4
        sl = slice(g * HL, (g + 1) * HL)
        in_maps.append({
            "xT": np.ascontiguousarray(x[b].T),
            "wq": np.ascontiguousarray(Wq[:, sl]),
            "wk": np.ascontiguousarray(Wk[:, sl]),
            "wv": np.ascontiguousarray(Wv[:, sl]),
            "wo": np.ascontiguousarray(Wo[sl, :]),
            "cosx": cosx,
            "sinx": sinx,
        })
    return in_maps


def gather(results):
    out = np.empty((B, S, E), dtype=np.float32)
    for b in range(B):
        acc = results[4 * b]["out"].astype(np.float32).copy()
        for g in range(1, 4):
            acc += results[4 * b + g]["out"]
        out[b] = acc
    return out


def kernel(x, Wq, Wk, Wv, Wo):
    from concourse.bass_utils import run_bass_kernel_spmd

    nc = get_nc()
    in_maps = make_in_maps(
        np.asarray(x), np.asarray(Wq), np.asarray(Wk), np.asarray(Wv), np.asarray(Wo)
    )
    res = run_bass_kernel_spmd(nc, in_maps, list(range(NCORES)))
    return gather(res.results)


# revision 5
# speedup vs baseline: 1.0924x; 1.0924x over previous
"""Multi-head causal attention with RoPE on 8 Trainium2 NeuronCores.

Sharding: tensor-parallel over heads x data-parallel over batch.
Core c handles batch b = c//4 and heads [4*(c%4), 4*(c%4)+4) (Hl=256 of Hd=1024).
Each core computes q/k/v projections for its head slice (column-split Wq/Wk/Wv),
RoPE, causal softmax attention, and a partial output projection (row-split Wo).
The host sums the 4 partial outputs per batch (the "all-reduce").

Device layouts (per core, S=2048, E=1024, Hl=256, D=64):
  xT   [E, S]    x transposed (host-side) so E rides the partition dim
  qT/kT slabs [128, S] x2: partitions = 2 heads x 64 dims, free = seq
  v    16 tiles [128, 260]: partitions = seq chunk, free = 4 heads x (64 dims + ones col)
  scores computed transposed (keys on partitions), softmax Z via ones-column of v,
  normalization by 1/Z broadcast via a DRAM-roundtrip partition-broadcast DMA.

All matmuls run in float32r (single-pass PE, ~1.5e-4 rel err measured on HW).
Attention processes the two heads of a slab in lockstep: their K=64 score
matmuls land on disjoint PE row groups (base partitions 0/64) and overlap,
and the interleave keeps the PE fed while ACT computes exp.
"""
import sys

sys.path.insert(0, "/opt/trn_rl_repo")
import numpy as np  # noqa: E402

N_HEADS = 16
B, S, E, HD = 2, 2048, 1024, 1024
D = HD // N_HEADS  # 64
HPC = 4            # heads per core
HL = HPC * D       # 256
NCORES = 8
ROPE_BASE = 10000.0

_built = None


def _build_nc():
    import concourse.bass as bass
    import concourse.tile as tile
    from concourse import bacc, mybir

    F32 = mybir.dt.float32
    F32R = mybir.dt.float32r
    Exp = mybir.ActivationFunctionType.Exp
    is_ge = mybir.AluOpType.is_ge
    ts = bass.ts

    nc = bacc.Bacc("TRN2", target_bir_lowering=False, debug=False)
    xT_d = nc.dram_tensor("xT", [E, S], F32, kind="ExternalInput").ap()
    wq_d = nc.dram_tensor("wq", [E, HL], F32, kind="ExternalInput").ap()
    wk_d = nc.dram_tensor("wk", [E, HL], F32, kind="ExternalInput").ap()
    wv_d = nc.dram_tensor("wv", [E, HL], F32, kind="ExternalInput").ap()
    wo_d = nc.dram_tensor("wo", [HL, E], F32, kind="ExternalInput").ap()
    cos_d = nc.dram_tensor("cosx", [128, S], F32, kind="ExternalInput").ap()
    sin_d = nc.dram_tensor("sinx", [128, S], F32, kind="ExternalInput").ap()
    out_d = nc.dram_tensor("out", [S, E], F32, kind="ExternalOutput").ap()
    zscr_d = nc.dram_tensor("zscr", [HPC, S], F32).ap()  # internal scratch

    ECH = E // 128   # 8 e-chunks
    SCH = S // 128   # 16 seq chunks
    SB = S // 512    # 4 seq blocks
    swap_mask = []
    for i in range(16):
        swap_mask += [2 * i + 1, 2 * i]

    with tile.TileContext(nc) as tc:
        with (
            tc.tile_pool(name="persist", bufs=1) as pp,
            tc.tile_pool(name="evict", bufs=3) as ev,
        ):
            # persistent tiles
            qT = [pp.tile([128, S], F32R, tag=f"qT{c}", name=f"qT{c}") for c in range(2)]
            kT = [pp.tile([128, S], F32R, tag=f"kT{c}", name=f"kT{c}") for c in range(2)]
            vt = [pp.tile([128, HPC * (D + 1)], F32R, tag=f"v{t}", name=f"v{t}")
                  for t in range(SCH)]
            oT = [pp.tile([128, S], F32R, tag=f"oT{c}", name=f"oT{c}") for c in range(2)]
            cosx = pp.tile([128, S], F32R, tag="cosx", name="cosx")
            sinx = pp.tile([128, S], F32R, tag="sinx", name="sinx")
            wo_t = pp.tile([128, 2, E], F32R, tag="wo", name="wo")

            # small/constant loads on the scalar queue, weights spread across
            # queues so nothing waits behind the big xT stream
            nc.scalar.dma_start(out=cosx[:], in_=cos_d.bitcast(F32R))
            nc.scalar.dma_start(out=sinx[:], in_=sin_d.bitcast(F32R))
            nc.scalar.dma_start(
                out=wo_t[:], in_=wo_d.rearrange("(c p) e -> p c e", p=128).bitcast(F32R)
            )

            # ---------------- Phase B: projections + RoPE ----------------
            with (
                tc.tile_pool(name="bx", bufs=1) as bx,
                tc.tile_pool(name="bswp", bufs=2) as bswp,
                tc.tile_pool(name="bps", bufs=4, space="PSUM") as bps,
            ):
                wq_t = bx.tile([128, ECH, HL], F32R, tag="wq", name="wq")
                wk_t = bx.tile([128, ECH, HL], F32R, tag="wk", name="wk")
                wv_t = bx.tile([128, ECH, HL], F32R, tag="wv", name="wv")
                for w_d_, w_t_, eng in (
                    (wq_d, wq_t, nc.sync),
                    (wk_d, wk_t, nc.gpsimd),
                    (wv_d, wv_t, nc.scalar),
                ):
                    eng.dma_start(
                        out=w_t_[:],
                        in_=w_d_.rearrange("(c p) m -> p c m", p=128).bitcast(F32R),
                    )
                xt = []
                dma_engs = [nc.sync, nc.gpsimd, nc.scalar]
                for e in range(ECH):
                    xe = bx.tile([128, S], F32R, tag=f"x{e}", name=f"x{e}")
                    dma_engs[e % 3].dma_start(
                        out=xe[:], in_=xT_d[e * 128:(e + 1) * 128, :].bitcast(F32R)
                    )
                    xt.append(xe)

                # q/k projections -> transposed slabs
                for w_t_, dest in ((wq_t, qT), (wk_t, kT)):
                    for m in range(2):
                        for j in range(SB):
                            ps = bps.tile([128, 512], F32, tag="mm", name="mm")
                            for e in range(ECH):
                                nc.tensor.matmul(
                                    ps[:],
                                    w_t_[:, e, m * 128:(m + 1) * 128],
                                    xt[e][:, ts(j, 512)],
                                    start=(e == 0),
                                    stop=(e == ECH - 1),
                                )
                            nc.vector.tensor_copy(
                                out=dest[m][:, ts(j, 512)], in_=ps[:]
                            )
                # RoPE right away so DVE overlaps the v projection
                for dest in (qT, kT):
                    for c in range(2):
                        sw = bswp.tile([128, S], F32R, tag="swp", name="swp")
                        nc.vector.stream_shuffle(
                            out=sw[:].bitcast(F32),
                            in_=dest[c][:].bitcast(F32),
                            mask=swap_mask,
                        )
                        nc.vector.tensor_mul(out=sw[:], in0=sw[:], in1=sinx[:])
                        nc.vector.tensor_mul(
                            out=dest[c][:], in0=dest[c][:], in1=cosx[:]
                        )
                        nc.vector.tensor_add(
                            out=dest[c][:], in0=dest[c][:], in1=sw[:]
                        )
                # v projection -> seq-partition tiles with ones column
                for t in range(SCH):
                    nc.gpsimd.memset(
                        vt[t].rearrange("p (h c) -> p h c", c=D + 1)[:, :, D:D + 1]
                        .bitcast(F32),
                        1.0,
                    )
                    ps = bps.tile([128, HL], F32, tag="mm", name="mmv")
                    for e in range(ECH):
                        nc.tensor.matmul(
                            ps[:],
                            xt[e][:, ts(t, 128)],
                            wv_t[:, e, :],
                            start=(e == 0),
                            stop=(e == ECH - 1),
                        )
                    nc.vector.tensor_copy(
                        out=vt[t].rearrange("p (h c) -> p h c", c=D + 1)[:, :, 0:D],
                        in_=ps.rearrange("p (h c) -> p h c", c=D),
                    )

            # ---------------- Phase C: attention, two heads in lockstep -------
            with (
                tc.tile_pool(name="cexp", bufs=4) as cexp,
                tc.tile_pool(name="cz", bufs=1) as cz,
                tc.tile_pool(name="crb", bufs=1) as crb,
                tc.tile_pool(name="csc", bufs=3, space="PSUM") as csc,
                tc.tile_pool(name="cpv", bufs=1, space="PSUM") as cpv,
            ):
                for c in range(2):  # slab = head pair (2c, 2c+1)
                    hs = [2 * c, 2 * c + 1]
                    qs = [qT[c][0:64, :], qT[c][64:128, :]]
                    ks = [kT[c][0:64, :], kT[c][64:128, :]]
                    vs = [
                        [vt[t].rearrange("p (h c) -> p h c", c=D + 1)[:, h, :]
                         for t in range(SCH)]
                        for h in hs
                    ]
                    zst = [cz.tile([65, S], F32, tag=f"zst{i}", name=f"zst{i}")
                           for i in range(2)]
                    for j in range(SB):
                        nt = 4 * (j + 1)
                        pv = [cpv.tile([65, 512], F32, tag=f"pv{i}", name=f"pv{i}")
                              for i in range(2)]
                        for tp in range(nt // 2):
                            sc = [csc.tile([128, 1024], F32, tag="sc", name="sc")
                                  for _ in range(2)]
                            # score matmuls: head 0 on PE rows 0-63, head 1 on
                            # rows 64-127 -> adjacent pairs overlap in the array
                            for half in range(2):
                                t = 2 * tp + half
                                for i in range(2):
                                    nc.tensor.matmul(
                                        sc[i][:, ts(half, 512)],
                                        ks[i][:, ts(t, 128)],
                                        qs[i][:, ts(j, 512)],
                                        start=True,
                                        stop=True,
                                    )
                            exm = []
                            for i in range(2):
                                ex = cexp.tile([128, 1024], F32R, tag="ex", name="ex")
                                nc.scalar.activation(
                                    out=ex[:], in_=sc[i][:], func=Exp, scale=0.125
                                )
                                exm.append(ex)
                            for half in range(2):
                                t = 2 * tp + half
                                if t >= nt - 4:  # diagonal chunk: causal mask
                                    for i in range(2):
                                        nc.gpsimd.affine_select(
                                            out=exm[i][:, ts(half, 512)],
                                            in_=exm[i][:, ts(half, 512)],
                                            compare_op=is_ge,
                                            fill=0.0,
                                            base=(j * 512 - t * 128),
                                            channel_multiplier=-1,
                                            pattern=[[1, 512]],
                                        )
                                for i in range(2):
                                    nc.tensor.matmul(
                                        pv[i][:],
                                        vs[i][t],
                                        exm[i][:, ts(half, 512)],
                                        start=(t == 0),
                                        stop=(t == nt - 1),
                                    )
                        for i in range(2):
                            nc.vector.tensor_copy(
                                out=oT[c][i * 64:(i + 1) * 64, ts(j, 512)],
                                in_=pv[i][0:64, :],
                            )
                            nc.vector.tensor_copy(
                                out=zst[i][64:65, ts(j, 512)], in_=pv[i][64:65, :]
                            )
                    for i in range(2):
                        nc.sync.dma_start(out=zscr_d[hs[i], :], in_=zst[i][64:65, :])
                    # normalize this slab now (overlaps the next slab's compute):
                    # broadcast Z across partitions, invert on 128 lanes, multiply
                    rb = crb.tile([128, S], F32, tag="rb", name="rb")
                    for i in range(2):
                        nc.sync.dma_start(
                            out=rb[i * 64:(i + 1) * 64, :],
                            in_=zscr_d[hs[i]:hs[i] + 1, :].to_broadcast((64, S)),
                        )
                    rbr = crb.tile([128, S], F32R, tag="rbr", name="rbr")
                    nc.vector.reciprocal(out=rbr[:].bitcast(F32), in_=rb[:])
                    nc.vector.tensor_mul(out=oT[c][:], in0=oT[c][:], in1=rbr[:])

            # ---------------- Phase D: output projection (row-split Wo) --------
            with tc.tile_pool(name="dps", bufs=4, space="PSUM") as dps:
                for t in range(SCH):
                    for n in range(2):
                        ps = dps.tile([128, 512], F32, tag="wo", name="wops")
                        for c in range(2):
                            nc.tensor.matmul(
                                ps[:],
                                oT[c][:, ts(t, 128)],
                                wo_t[:, c, ts(n, 512)],
                                start=(c == 0),
                                stop=(c == 1),
                            )
                        ot = ev.tile([128, 512], F32, tag="out", name="oev")
                        nc.vector.tensor_copy(out=ot[:], in_=ps[:])
                        nc.sync.dma_start(
                            out=out_d[ts(t, 128), ts(n, 512)], in_=ot[:]
                        )

    nc.compile()
    return nc


def _rope_tables():
    iexp = np.arange(0, D, 2, dtype=np.float32) / np.float32(D)
    inv_freq = np.reciprocal(np.power(np.float32(ROPE_BASE), iexp))  # (32,) f32
    ang = np.arange(S, dtype=np.float32)[:, None] * inv_freq[None, :]  # (S, 32)
    cos = np.cos(ang).astype(np.float32)  # (S, 32)
    sin = np.sin(ang).astype(np.float32)
    cosx = np.empty((64, S), dtype=np.float32)
    sinx = np.empty((64, S), dtype=np.float32)
    cosx[0::2] = cos.T
    cosx[1::2] = cos.T
    sinx[0::2] = -sin.T
    sinx[1::2] = sin.T
    return np.tile(cosx, (2, 1)), np.tile(sinx, (2, 1))  # (128, S) each


def get_nc():
    global _built
    if _built is None:
        _built = _build_nc()
    return _built


def make_in_maps(x, Wq, Wk, Wv, Wo):
    cosx, sinx = _rope_tables()
    in_maps = []
    for c in range(NCORES):
        b, g = c // 4, c % 4
        sl = slice(g * HL, (g + 1) * HL)
        in_maps.append({
            "xT": np.ascontiguousarray(x[b].T),
            "wq": np.ascontiguousarray(Wq[:, sl]),
            "wk": np.ascontiguousarray(Wk[:, sl]),
            "wv": np.ascontiguousarray(Wv[:, sl]),
            "wo": np.ascontiguousarray(Wo[sl, :]),
            "cosx": cosx,
            "sinx": sinx,
        })
    return in_maps


def gather(results):
    out = np.empty((B, S, E), dtype=np.float32)
    for b in range(B):
        acc = results[4 * b]["out"].astype(np.float32).copy()
        for g in range(1, 4):
            acc += results[4 * b + g]["out"]
        out[b] = acc
    return out


def kernel(x, Wq, Wk, Wv, Wo):
    from concourse.bass_utils import run_bass_kernel_spmd

    nc = get_nc()
    in_maps = make_in_maps(
        np.asarray(x), np.asarray(Wq), np.asarray(Wk), np.asarray(Wv), np.asarray(Wo)
    )
    res = run_bass_kernel_spmd(nc, in_maps, list(range(NCORES)))
    return gather(res.results)
